# revision 5
# baseline (speedup 1.0000x reference)
"""DoRA linear kernel for 8 Trainium2 NeuronCores.

out = (base_output + 2.0 * x @ lora_A^T @ lora_B^T) * magnitude / (||base_weight + 2.0 * lora_B @ lora_A||_row + eps)

Sharding (per the row-parallel hint):
  - tokens (B*S = 8192) data-parallel: 1024 per core (x, base_output, out)
  - base_weight / lora_B / magnitude row-parallel: 512 out_features per core
    (per-row norm fully local; mag_scale allgathered, 1KB bf16 collective)
  - lora_A and lora_B replicated for the activation path

v2 design notes (vs the 217us baseline):
  - x is transposed on the HOST into [128 d-part, tb, dc, 128 t] bf16 layout, so
    the xa = A @ x^T contraction streams straight off contiguous DMA loads.
    This removes all PE transposes and the scalar PSUM->SBUF copies of x.
  - base_output, the output, and the mag broadcast run in bf16 (halves HBM
    traffic; absmax rel err ~7.7e-3 measured vs the 2e-2 gate).
  - The row-norm path (stage 0), the mag tail, and the AllGather are emitted
    FIRST in every engine stream, so the collective's ~10-40us latency overlaps
    the main loop instead of gating the epilogue at ~95us.
  - Epilogue: delta(PSUM fp32) + base(bf16) -> osb bf16 on DVE; osb *= magb
    (bf16, 2x DVE rate) split DVE/GpSimd; stores alternate the two HWDGE rings.

Engine / DMA-ring assignment (each engine's instruction stream is FIFO):
  - sync  (SP)  ring: W0, W1, magnitude, a16, b2ft, b2st, base tiles, even stores
  - scalar(ACT) ring: W2, W3, a_t, xt tiles, odd stores
  - gpsimd SWDGE:     cc in/out + mag partition-broadcast, 2/8 of the mults
  - vector:           norm reduces, mag tail, epilogue adds, 6/8 of the mults
  - scalar compute:   norm squares, sqrt, xa PSUM->SBUF copies
"""

import sys

sys.path.insert(0, "/opt/trn_rl_repo")

import ml_dtypes
import numpy as np

import concourse.bass as bass  # noqa: F401
import concourse.mybir as mybir
import concourse.tile as tile
from concourse import bacc
from concourse.bass_utils import run_bass_kernel_spmd
from concourse.masks import make_identity

N_CORES = 8
T, D, O, R = 8192, 4096, 4096, 64
T_LOC = T // N_CORES  # 1024 tokens per core
O_SH = O // N_CORES  # 512 weight rows per core
SCALING = 2.0
EPS = 1e-8
F32 = mybir.dt.float32
BF16 = mybir.dt.bfloat16
NP_BF16 = ml_dtypes.bfloat16

N_TB = T_LOC // 128  # 8 token blocks per core
N_OC = O_SH // 128  # 4 o-chunks per core (stage 0)
N_DC512 = D // 512  # 8 d-chunks of 512
N_DC128 = D // 128  # 32 d-chunks of 128

_CACHE: dict = {}


def _emit(nc, tc, aps):
    xt_d = aps["xt_shard"]
    base_d = aps["base_shard"]
    w_d = aps["w_shard"]
    b_sh_d = aps["b_shard"]
    b_full_d = aps["b_full"]
    a_d = aps["a_full"]
    at_d = aps["a_t"]
    mag_d = aps["mag_shard"]
    out_d = aps["out_shard"]

    import contextlib

    ctx = contextlib.ExitStack()
    with ctx:
        const = ctx.enter_context(tc.tile_pool(name="const", bufs=1))
        wpool = ctx.enter_context(tc.tile_pool(name="wpool", bufs=4))
        xtpool = ctx.enter_context(tc.tile_pool(name="xtpool", bufs=3))
        bpool = ctx.enter_context(tc.tile_pool(name="bpool", bufs=3))
        xapool = ctx.enter_context(tc.tile_pool(name="xapool", bufs=2))
        opool = ctx.enter_context(tc.tile_pool(name="opool", bufs=8))
        sqpool = ctx.enter_context(tc.tile_pool(name="sqpool", bufs=3))
        scpool = ctx.enter_context(tc.tile_pool(name="scpool", bufs=2))
        p_u = ctx.enter_context(tc.tile_pool(name="p_u", bufs=4, space="PSUM"))
        p_xa = ctx.enter_context(tc.tile_pool(name="p_xa", bufs=1, space="PSUM"))
        p_o = ctx.enter_context(tc.tile_pool(name="p_o", bufs=3, space="PSUM"))
        dram = ctx.enter_context(tc.tile_pool(name="dram", bufs=1, space="DRAM"))

        ident = const.tile([128, 128], BF16)
        make_identity(nc, ident[:])

        base_r = base_d.rearrange("(tb p) d -> tb p d", p=128)
        out_r = out_d.rearrange("(tb p) d -> tb p d", p=128)
        w_r = w_d.rearrange("(oc p) d -> oc p d", p=128)
        xt_r = xt_d.rearrange("p (tb f) -> tb p f", tb=N_TB)

        # ---- phase A: DMA triggers. Norm-path inputs go FIRST so stage 0
        # is ready early; x/base interleave in consumption order behind them.
        # sync ring: b2st, a16, W0, W1, mag, then (xt_tb, base_tb) pairs
        b2st_sb = const.tile([R, O_SH], BF16)
        nc.sync.dma_start(b2st_sb[:], b_sh_d[:])
        a16_sb = const.tile([R, D], BF16)
        nc.sync.dma_start(a16_sb[:], a_d[:])
        w_tiles = []
        for oc in range(N_OC):
            wt = wpool.tile([128, D], BF16, tag="w", name=f"w_{oc}")
            eng = nc.sync if oc < 2 else nc.scalar
            eng.dma_start(wt[:], w_r[oc])
            w_tiles.append(wt)
        magsh_sb = const.tile([128, N_OC], F32)
        nc.sync.dma_start(magsh_sb[:], mag_d.rearrange("(oc p) -> p oc", p=128))

        # scalar ring: W2, W3 (above), a_t, b2ft -- no pool-gated loads here,
        # so scalar-engine compute (squares, xa copies) is never FIFO-jammed.
        at_sb = const.tile([128, N_DC128 * R], BF16)
        nc.scalar.dma_start(at_sb[:], at_d[:])
        b2ft_sb = const.tile([R, O], BF16)
        nc.scalar.dma_start(b2ft_sb[:], b_full_d[:])

        xt_tiles = {}
        base_tiles = {}

        def load_xt(tb):
            t = xtpool.tile([128, N_DC128 * 128], BF16, tag="xt", name=f"xt_{tb}")
            nc.sync.dma_start(t[:], xt_r[tb])
            xt_tiles[tb] = t

        def load_base(tb):
            bt = bpool.tile([128, O], BF16, tag="base", name=f"base_{tb}")
            nc.sync.dma_start(bt[:], base_r[tb])
            base_tiles[tb] = bt

        for tb in range(N_TB):
            load_xt(tb)
            load_base(tb)

        # ---- stage 0: ||W + 2 B A||^2 rows -> mag_scale -> allgather.
        # Emitted first in the PE/scalar/vector/gpsimd streams so the
        # collective latency overlaps the main loop. Each Square also
        # row-sums via accum_out, so no big DVE reduce is needed.
        part_sb = const.tile([128, N_OC, N_DC512], F32)
        ss_sb = const.tile([128, N_OC, 1], F32)
        magsc16_sb = const.tile([128, N_OC], BF16)
        magb_sb = const.tile([128, O], BF16)

        for oc in range(N_OC):
            for half in range(2):
                pus = []
                for j in range(4):
                    dc = 4 * half + j
                    pu = p_u.tile([128, 512], F32, tag="pu", name=f"pu_{oc}_{dc}")
                    nc.tensor.matmul(
                        pu[:],
                        b2st_sb[:, 128 * oc : 128 * (oc + 1)],
                        a16_sb[:, 512 * dc : 512 * (dc + 1)],
                        start=True,
                        stop=False,
                    )
                    pus.append(pu)
                for j in range(4):
                    dc = 4 * half + j
                    nc.tensor.matmul(
                        pus[j][:],
                        ident[:],
                        w_tiles[oc][:, 512 * dc : 512 * (dc + 1)],
                        start=False,
                        stop=True,
                    )
                sqd = sqpool.tile(
                    [128, 4, 512], BF16, tag="sqd", name=f"sqd_{oc}_{half}"
                )
                for j in range(4):
                    dc = 4 * half + j
                    nc.scalar.activation(
                        sqd[:, j, :],
                        pus[j][:],
                        mybir.ActivationFunctionType.Square,
                        accum_out=part_sb[:, oc, dc : dc + 1],
                    )
            nc.vector.tensor_reduce(
                ss_sb[:, oc, :],
                part_sb[:, oc, :],
                axis=mybir.AxisListType.X,
                op=mybir.AluOpType.add,
            )

        # mag tail: magsc = magnitude / (sqrt(ss) + eps), cast bf16
        for oc in range(N_OC):
            nrm = scpool.tile([128, 1], F32, tag="nrm", name=f"nrm_{oc}")
            nc.scalar.sqrt(nrm[:], ss_sb[:, oc, :])
            nc.vector.tensor_scalar_add(nrm[:], nrm[:], EPS)
            rinv = scpool.tile([128, 1], F32, tag="rinv", name=f"rinv_{oc}")
            nc.vector.reciprocal(rinv[:], nrm[:])
            nc.vector.tensor_tensor(
                out=magsc16_sb[:, oc : oc + 1],
                in0=rinv[:],
                in1=magsh_sb[:, oc : oc + 1],
                op=mybir.AluOpType.mult,
            )

        # collective + partition broadcast, all on the gpsimd ring
        cc_in = dram.tile([O_SH], BF16)
        cc_out = dram.tile([O], BF16, addr_space="Shared")
        nc.gpsimd.dma_start(cc_in.rearrange("(oc p) -> p oc", p=128), magsc16_sb[:])
        nc.gpsimd.collective_compute(
            "AllGather",
            mybir.AluOpType.bypass,
            replica_groups=[list(range(N_CORES))],
            ins=[cc_in[:]],
            outs=[cc_out[:]],
        )
        nc.gpsimd.dma_start(magb_sb[:], cc_out[None, :].partition_broadcast(128))

        # ---- main loop: xa (K-accumulated over 32 d-chunks) then delta + add
        osb_tiles = {}
        for tb in range(N_TB):
            xh = xt_tiles.pop(tb)
            pxa = p_xa.tile([R, 128], F32, tag="pxa", name=f"pxa_{tb}")
            for dc in range(N_DC128):
                nc.tensor.matmul(
                    pxa[:],
                    at_sb[:, R * dc : R * (dc + 1)],
                    xh[:, 128 * dc : 128 * (dc + 1)],
                    start=(dc == 0),
                    stop=(dc == N_DC128 - 1),
                )
            xa_sb = xapool.tile([R, 128], BF16, tag="xa", name=f"xa_{tb}")
            nc.scalar.copy(xa_sb[:], pxa[:])

            osb = opool.tile([128, O], BF16, tag="o", name=f"osb_{tb}")
            osb_tiles[tb] = osb
            for h in range(2):
                pos = [
                    p_o.tile([128, 512], F32, tag="po", name=f"po_{tb}_{h}_{j}")
                    for j in range(4)
                ]
                for j in range(4):
                    och = 4 * h + j
                    nc.tensor.matmul(
                        pos[j][:],
                        xa_sb[:],
                        b2ft_sb[:, 512 * och : 512 * (och + 1)],
                        start=True,
                        stop=True,
                    )
                bh = base_tiles[tb]
                for j in range(4):
                    och = 4 * h + j
                    nc.vector.tensor_tensor(
                        out=osb[:, 512 * och : 512 * (och + 1)],
                        in0=pos[j][:],
                        in1=bh[:, 512 * och : 512 * (och + 1)],
                        op=mybir.AluOpType.add,
                    )
                if h == 1:
                    del base_tiles[tb]

        # ---- epilogue: magnitude rescale (bf16, 2x DVE rate) + stores
        for tb in range(N_TB):
            osb = osb_tiles[tb]
            eng = nc.gpsimd if tb in (2, 6) else nc.vector
            for h in range(2):
                eng.tensor_tensor(
                    out=osb[:, 2048 * h : 2048 * (h + 1)],
                    in0=osb[:, 2048 * h : 2048 * (h + 1)],
                    in1=magb_sb[:, 2048 * h : 2048 * (h + 1)],
                    op=mybir.AluOpType.mult,
                )
            eng_dma = nc.sync if tb % 2 == 0 else nc.scalar
            eng_dma.dma_start(out_r[tb], osb[:])


def _build():
    nc = bacc.Bacc(
        "TRN2", target_bir_lowering=False, debug=False, num_devices=N_CORES
    )
    aps = {
        "xt_shard": nc.dram_tensor(
            "xt_shard", [128, N_TB * N_DC128 * 128], BF16, kind="ExternalInput"
        ).ap(),
        "base_shard": nc.dram_tensor(
            "base_shard", [T_LOC, O], BF16, kind="ExternalInput"
        ).ap(),
        "w_shard": nc.dram_tensor("w_shard", [O_SH, D], BF16, kind="ExternalInput").ap(),
        "b_shard": nc.dram_tensor("b_shard", [R, O_SH], BF16, kind="ExternalInput").ap(),
        "b_full": nc.dram_tensor("b_full", [R, O], BF16, kind="ExternalInput").ap(),
        "a_full": nc.dram_tensor("a_full", [R, D], BF16, kind="ExternalInput").ap(),
        "a_t": nc.dram_tensor(
            "a_t", [128, N_DC128 * R], BF16, kind="ExternalInput"
        ).ap(),
        "mag_shard": nc.dram_tensor(
            "mag_shard", [O_SH], F32, kind="ExternalInput"
        ).ap(),
        "out_shard": nc.dram_tensor(
            "out_shard", [T_LOC, O], BF16, kind="ExternalOutput"
        ).ap(),
    }
    with tile.TileContext(nc) as tc:
        _emit(nc, tc, aps)
    nc.compile()
    return nc


def run(inputs: dict, trace: bool = False):
    """Run the SPMD kernel on full inputs; returns (full_output, BassKernelResults)."""
    if "nc" not in _CACHE:
        _CACHE["nc"] = _build()
    nc = _CACHE["nc"]

    x = np.asarray(inputs["x"], dtype=np.float32).reshape(T, D).astype(NP_BF16)
    base = (
        np.asarray(inputs["base_output"], dtype=np.float32)
        .reshape(T, O)
        .astype(NP_BF16)
    )
    w = np.asarray(inputs["base_weight"], dtype=np.float32).astype(NP_BF16)
    a = np.ascontiguousarray(
        (np.asarray(inputs["lora_A"], dtype=np.float32) * SCALING).astype(NP_BF16)
    )
    # a_t[p, dc*R + r] = 2*A[r, dc*128 + p]
    at = np.ascontiguousarray(
        a.T.reshape(N_DC128, 128, R).transpose(1, 0, 2).reshape(128, N_DC128 * R)
    )
    bt = np.asarray(inputs["lora_B"], dtype=np.float32).astype(NP_BF16).T
    mag = np.asarray(inputs["magnitude"], dtype=np.float32)

    in_maps = []
    for c in range(N_CORES):
        xs = x[c * T_LOC : (c + 1) * T_LOC]
        # xt[p, tb*4096 + dc*128 + tloc] = x[tb*128 + tloc, dc*128 + p]
        xt = np.ascontiguousarray(
            xs.reshape(N_TB, 128, N_DC128, 128)
            .transpose(3, 0, 2, 1)
            .reshape(128, N_TB * N_DC128 * 128)
        )
        in_maps.append(
            {
                "xt_shard": xt,
                "base_shard": np.ascontiguousarray(base[c * T_LOC : (c + 1) * T_LOC]),
                "w_shard": np.ascontiguousarray(w[c * O_SH : (c + 1) * O_SH]),
                "b_shard": np.ascontiguousarray(bt[:, c * O_SH : (c + 1) * O_SH]),
                "b_full": np.ascontiguousarray(bt),
                "a_full": a,
                "a_t": at,
                "mag_shard": np.ascontiguousarray(mag[c * O_SH : (c + 1) * O_SH]),
            }
        )

    res = run_bass_kernel_spmd(
        nc, in_maps, core_ids=list(range(N_CORES)), trace=trace
    )
    out = np.concatenate(
        [
            np.asarray(res.results[c]["out_shard"]).astype(np.float32)
            for c in range(N_CORES)
        ],
        axis=0,
    )
    return out, res


def kernel(**inputs) -> np.ndarray:
    x = inputs["x"]
    out, _ = run(inputs)
    return out.reshape(x.shape[0], x.shape[1], O).astype(np.float32)
